# revision 54
# baseline (speedup 1.0000x reference)
"""Trainium2 Bass kernel for the Mamba-1 block (nn_Block_9122510537354).

Self-contained: hardcodes shapes/sharding. d_inner is sharded over 8 cores;
each core computes a partial out_proj contribution which the host sums.
The kernel returns (out, residual) matching reference.reference().

Design notes (engine assignment chosen against the CoreSim timing model,
within the real TRN2 backend constraints: GPSIMD has no PSUM access and no
tensor_tensor_scan/scalar_tensor_tensor; DMA cannot read PSUM):
- scan section: dA exps on Scalar, dBx mults on GpSimd, the 64 sequential
  scans on DVE (the only engine that can scan), C-mult split DVE/GpSimd,
  and the state-reduction as identity-matmuls accumulated in PSUM on PE
  (replaces the old DVE add tree). D*x is folded in via a diag(D) matmul;
  silu(z) gating is one DVE mult straight out of PSUM.
- depthwise conv runs on PE as 4 shifted diag(w_k) matmuls into PSUM.
- B/C rows broadcast to 128 partitions with batched DMAs per (b, g)
  reading the AllReduce output in DRAM directly.
- ONE bf16 AllReduce for all 96 x_proj rows (collective count matters
  far more than payload on HW); the in_proj z-half is emitted after the
  collective launch so PE overlaps it.
- h/res stream in as bf16 (halves input DMA), out partials stored bf16
  and summed in f32 on the host.
- LN: wave-scheduled per 4-tile group (loads -> stats -> sqrt -> recip ->
  fused (rn-mu)*rstd on GpSimd -> PE transposes) to avoid cross-engine
  dependency ladders on the in-order queues.
- software pipeline: unit i+1's dA/dBx/scans are emitted before unit i's
  C-mult so no engine queue drains; triple-buffered slabs.
"""
import contextlib
import time
import numpy as np

import concourse.tile as tile
import concourse.mybir as mybir
from concourse.vector_clock import ScopedClock


def _patched_drain_and_barrier(self, tick_clock, wait_clock):
    nc = self.nc
    probe = nc.sync.nop(nofuse=True, hint="drain_waits")
    wait_clock.add_sem_waits(probe.ins, ScopedClock({None: tick_clock.global_clock}))
    waits = list(probe.ins.sync_info.on_wait)
    if len(waits) > 1:
        probe.ins.sync_info.on_wait[:] = waits[:1]
        for w in waits[1:]:
            extra = nc.sync.nop(nofuse=True, hint="drain_waits")
            extra.ins.sync_info = mybir.SyncInfo(on_wait=[w], on_update=[])
    nc.sync.drain()
    nc.all_engine_barrier()
    assert self.sems is not None
    popped = nc._tile_sem_poison_stack.pop()
    assert popped is self._sem_poison
    nc.clear_and_free_semaphores(list(self.sems.allocated().values()))
    nc.all_engine_barrier()


tile.TileContext._drain_and_barrier = _patched_drain_and_barrier

_split_ctr = [0]


def split_multiwaits(nc):
    """Mutate nc.m so no instruction carries >1 sync wait."""
    n_split = 0
    for fn in nc.m.functions:
        for blk in fn.blocks:
            insts = blk.instructions
            i = 0
            while i < len(insts):
                inst = insts[i]
                si = getattr(inst, "sync_info", None)
                if si is not None and si.on_wait and len(si.on_wait) > 1:
                    waits = list(si.on_wait)
                    si.on_wait[:] = waits[:1]
                    new_nops = []
                    for w in waits[1:]:
                        _split_ctr[0] += 1
                        new_nops.append(
                            mybir.InstNoOp(
                                name=f"I-mwsplit-{_split_ctr[0]}",
                                engine=inst.engine,
                                bass_nofuse=True,
                                sync_info=mybir.SyncInfo(on_wait=[w], on_update=[]),
                            )
                        )
                    insts[i:i] = new_nops
                    i += len(new_nops)
                    n_split += 1
                i += 1
    return n_split


import concourse.bass as bass
import concourse.tile as tile
import concourse.mybir as mybir

dt = mybir.dt
AF = mybir.ActivationFunctionType
ALU = mybir.AluOpType

B, L, DM = 2, 1024, 1024
DI, S, R, KCONV = 2048, 16, 64, 4
NCORES = 8
DLOC = DI // NCORES          # 256
TOK = B * L                  # 2048
NTT = TOK // 128             # 16 token tiles
NDT = DLOC // 128            # 2 channel tiles
SG = 8                       # states per group
NSG = S // SG                # 2 groups
EPS = 1e-5
LPAD = L + 4                 # per-b padded conv row

f32, bf16 = dt.float32, dt.bfloat16


def build(nc, n_cores=NCORES):
    # ---------------- DRAM I/O ----------------
    h_d = nc.dram_tensor("h", [TOK, DM], bf16, kind="ExternalInput")
    res_d = nc.dram_tensor("res", [TOK, DM], bf16, kind="ExternalInput")
    W_in_d = nc.dram_tensor("w_in", [DM, 2 * DLOC], bf16, kind="ExternalInput")
    in_b_d = nc.dram_tensor("in_b", [2 * DLOC], f32, kind="ExternalInput")
    convw_d = nc.dram_tensor("convw", [NDT * KCONV * 128, 128], bf16, kind="ExternalInput")
    convb_d = nc.dram_tensor("convb", [DLOC], f32, kind="ExternalInput")
    xp_d = nc.dram_tensor("xp", [DLOC, 96], bf16, kind="ExternalInput")
    dtp_d = nc.dram_tensor("dtp", [R, DLOC], bf16, kind="ExternalInput")
    dtb_d = nc.dram_tensor("dtb", [DLOC], f32, kind="ExternalInput")
    A_d = nc.dram_tensor("A", [DLOC, S], f32, kind="ExternalInput")
    diagD_d = nc.dram_tensor("diagD", [DLOC, 128], bf16, kind="ExternalInput")
    op_d = nc.dram_tensor("op", [DLOC, DM], bf16, kind="ExternalInput")
    ident_d = nc.dram_tensor("ident", [128, 128], bf16, kind="ExternalInput")

    out_d = nc.dram_tensor("out_part", [TOK, DM], bf16, kind="ExternalOutput")

    hv = h_d[:].rearrange("(n p) d -> n p d", p=128)
    rv = res_d[:].rearrange("(n p) d -> n p d", p=128)
    outv = out_d[:].rearrange("(n p) d -> n p d", p=128)

    _res = _build_body(nc, locals())
    split_multiwaits(nc)
    return _res


def _build_body(nc, T):
    h_d, res_d, W_in_d, in_b_d = T["h_d"], T["res_d"], T["W_in_d"], T["in_b_d"]
    convw_d, convb_d, xp_d, dtp_d = T["convw_d"], T["convb_d"], T["xp_d"], T["dtp_d"]
    dtb_d, A_d, diagD_d, op_d, ident_d = T["dtb_d"], T["A_d"], T["diagD_d"], T["op_d"], T["ident_d"]
    out_d, hv, rv, outv = T["out_d"], T["hv"], T["rv"], T["outv"]
    n_cores = T["n_cores"]
    with tile.TileContext(nc) as tc, contextlib.ExitStack() as ctx:
        const = ctx.enter_context(tc.tile_pool(name="const", bufs=1))
        live = ctx.enter_context(tc.tile_pool(name="live", bufs=1))
        small = ctx.enter_context(tc.tile_pool(name="small", bufs=6))
        dram = ctx.enter_context(tc.tile_pool(name="dram", bufs=1, space="DRAM"))

        # ---------------- constants ----------------
        in_bias = const.tile([128, (2 * DLOC) // 128], f32)
        nc.sync.dma_start(out=in_bias[:], in_=in_b_d[:].rearrange("(ft p) -> p ft", p=128))
        convdiag = const.tile([128, NDT, KCONV, 128], bf16)
        nc.scalar.dma_start(out=convdiag[:],
                            in_=convw_d[:].rearrange("(dtl k p) f -> p dtl k f", p=128, k=KCONV))
        convb = const.tile([128, NDT], f32)
        nc.sync.dma_start(out=convb[:], in_=convb_d[:].rearrange("(dtl p) -> p dtl", p=128))
        xpw = const.tile([128, NDT, 96], bf16)
        nc.scalar.dma_start(out=xpw[:], in_=xp_d[:].rearrange("(dtl p) f -> p dtl f", p=128))
        dtpw = const.tile([R, NDT, 128], bf16)
        nc.scalar.dma_start(out=dtpw[:], in_=dtp_d[:].rearrange("r (dtl p) -> r dtl p", p=128))
        dtb = const.tile([128, NDT], f32)
        nc.sync.dma_start(out=dtb[:], in_=dtb_d[:].rearrange("(dtl p) -> p dtl", p=128))
        Asb = const.tile([128, NDT, S], f32)
        nc.sync.dma_start(out=Asb[:], in_=A_d[:].rearrange("(dtl p) s -> p dtl s", p=128))
        diagD = const.tile([128, NDT, 128], bf16)
        nc.scalar.dma_start(out=diagD[:], in_=diagD_d[:].rearrange("(dtl p) f -> p dtl f", p=128))
        opw = const.tile([128, NDT, DM], bf16)
        nc.scalar.dma_start(out=opw[:], in_=op_d[:].rearrange("(dtl p) f -> p dtl f", p=128))
        ident = const.tile([128, 128], bf16)
        nc.scalar.dma_start(out=ident[:], in_=ident_d[:])
        epsb = const.tile([128, 1], f32)
        nc.vector.memset(epsb[:], EPS)
        oneb = const.tile([128, 1], f32)
        nc.vector.memset(oneb[:], 1.0)

        # persistent feature-major buffers
        szT = live.tile([128, NDT, TOK], bf16, tag="szT")
        xT = live.tile([128, NDT, TOK], bf16, tag="xT")
        dtT = live.tile([128, NDT, TOK], bf16, tag="dtT")
        uT = live.tile([128, NDT, TOK], bf16, tag="uT")
        yT = live.tile([128, NDT, TOK], bf16, tag="yT")
        dtl_bf = live.tile([R, TOK], bf16, tag="dtl_bf")

        # ========== P1-P3: add + LN + normalize + transpose ==========
        with tc.tile_pool(name="scope13", bufs=1) as scope13:
            hnT = scope13.tile([128, DM // 128, TOK], bf16, tag="hnT")
            W_in = scope13.tile([128, DM // 128, 2 * DLOC], bf16, tag="W_in")
            tq = [nc.sync, nc.scalar]
            hres = scope13.tile([128, NTT, 2, DM], bf16, tag="hres")
            for i in range(4):
                nc.sync.dma_start(out=hres[:, i, 0, :], in_=hv[i])
                nc.sync.dma_start(out=hres[:, i, 1, :], in_=rv[i])
            nc.sync.dma_start(out=W_in[:], in_=W_in_d[:].rearrange("(kt p) f -> p kt f", p=128))

            def load_chunk(c):
                nc.sync.dma_start(
                    out=hres[:, 4 * c:4 * (c + 1), 0, :],
                    in_=h_d[:].rearrange("(n p) d -> n p d", p=128)[4 * c:4 * (c + 1)]
                        .rearrange("n p d -> p n d"))
                nc.scalar.dma_start(
                    out=hres[:, 4 * c:4 * (c + 1), 1, :],
                    in_=res_d[:].rearrange("(n p) d -> n p d", p=128)[4 * c:4 * (c + 1)]
                        .rearrange("n p d -> p n d"))

            load_chunk(1)
            pending_tdma = []
            psum_t_stack = contextlib.ExitStack()
            psum_t = psum_t_stack.enter_context(
                tc.tile_pool(name="psum_t", bufs=2, space="PSUM"))

            def emit_transpose(i):
                pts = psum_t.tile([128, 8, 128], bf16, tag="tp", name=f"pt{i}")
                for j in range(8):
                    nc.tensor.transpose(pts[:, j, :],
                                        hres[:, i, 0, j * 128:(j + 1) * 128], ident[:])
                for j in range(8):
                    eng = nc.vector if (i + j) % 2 == 0 else nc.scalar
                    if (i + j) % 2 == 0:
                        nc.vector.tensor_copy(out=hnT[:, j, i * 128:(i + 1) * 128],
                                              in_=pts[:, j, :])
                    else:
                        nc.scalar.copy(out=hnT[:, j, i * 128:(i + 1) * 128],
                                       in_=pts[:, j, :])

            def flush_tdma():
                for i in pending_tdma:
                    emit_transpose(i)
                pending_tdma.clear()

            for grp in range(NTT // 4):
                gtiles = list(range(grp * 4, (grp + 1) * 4))
                rstds = {}
                for i in gtiles:
                    nc.gpsimd.tensor_add(out=hres[:, i, 1, :], in0=hres[:, i, 0, :],
                                         in1=hres[:, i, 1, :])
                    st = small.tile([128, 2, 6], f32, tag=f"st{i % 8}")
                    nc.vector.bn_stats(out=st[:, 0, :], in_=hres[:, i, 1, 0:512])
                    nc.vector.bn_stats(out=st[:, 1, :], in_=hres[:, i, 1, 512:1024])
                    mv = small.tile([128, 2], f32, tag=f"mv{i % 8}")
                    nc.vector.bn_aggr(out=mv[:], in_=st[:])
                    rstds[i] = mv
                for i in gtiles:
                    mv = rstds[i]
                    rstd = small.tile([128, 1], f32, tag=f"rstd{i % 8}")
                    nc.scalar.activation(out=rstd[:], in_=mv[:, 1:2], func=AF.Sqrt, bias=epsb[:])
                    rstds[i] = (mv, rstd)
                for i in gtiles:
                    _, rstd = rstds[i]
                    nc.vector.reciprocal(out=rstd[:], in_=rstd[:])
                for i in gtiles:
                    mv, rstd = rstds[i]
                    nc.gpsimd.tensor_scalar(out=hres[:, i, 0, :], in0=hres[:, i, 1, :],
                                            scalar1=mv[:, 0:1], scalar2=rstd[:],
                                            op0=ALU.subtract, op1=ALU.mult)
                prev = pending_tdma[:]
                pending_tdma.clear()
                pending_tdma.extend(gtiles)
                for i in prev:
                    emit_transpose(i)
                if grp + 2 < 4:
                    load_chunk(grp + 2)
            flush_tdma()
            psum_t_stack.close()

            psum_stack = contextlib.ExitStack()
            psum = psum_stack.enter_context(tc.tile_pool(name="psum", bufs=2, space="PSUM"))
            # ========== P4a: in_proj x-half ==========
            with tc.tile_pool(name="scope45", bufs=1) as scope45:
                xpreT = scope45.tile([128, NDT, B, LPAD], bf16, tag="xpreT")
                for d in range(NDT):
                    for b in range(B):
                        nc.vector.memset(xpreT[:, d, b, 0:4], 0.0)

                def in_proj_fq(fq):
                    for tci in range(4):   # token chunks of 512; 0,1 -> b0; 2,3 -> b1
                        b, off = divmod(tci, 2)
                        ps = psum.tile([128, 512], f32, tag="mm")
                        for k in range(8):
                            nc.tensor.matmul(ps[:], W_in[:, k, fq * 128:(fq + 1) * 128],
                                             hnT[:, k, tci * 512:(tci + 1) * 512],
                                             start=(k == 0), stop=(k == 7))
                        if fq < NDT:
                            nc.scalar.activation(
                                out=xpreT[:, fq, b, 4 + off * 512:4 + (off + 1) * 512],
                                in_=ps[:], func=AF.Identity, bias=in_bias[:, fq:fq + 1])
                        else:
                            nc.scalar.activation(
                                out=szT[:, fq - NDT, tci * 512:(tci + 1) * 512],
                                in_=ps[:], func=AF.Silu, bias=in_bias[:, fq:fq + 1])

                for fq in range(NDT):
                    in_proj_fq(fq)
                # ========== P5: conv (PE diag matmuls) + silu ==========
                for d in range(NDT):
                    for b in range(B):
                        for h in range(2):
                            cps = psum.tile([128, 512], f32, tag="convp")
                            for k in range(KCONV):
                                nc.tensor.matmul(
                                    cps[:], convdiag[:, d, k, :],
                                    xpreT[:, d, b, 1 + k + h * 512:1 + k + h * 512 + 512],
                                    start=(k == 0), stop=(k == KCONV - 1))
                            nc.scalar.activation(
                                out=xT[:, d, b * L + h * 512:b * L + (h + 1) * 512],
                                in_=cps[:], func=AF.Silu, bias=convb[:, d:d + 1])

                # ========== P6: x_proj partial + AllReduce (launch) ==========
                xdbl_part = scope45.tile([96, TOK], bf16, tag="xdblp")
                for tci in range(4):
                    ps = psum.tile([96, 512], f32, tag="xdbl")
                    for d in range(NDT):
                        nc.tensor.matmul(ps[:], xpw[:, d, :], xT[:, d, tci * 512:(tci + 1) * 512],
                                         start=(d == 0), stop=(d == NDT - 1))
                    nc.vector.tensor_copy(out=xdbl_part[:, tci * 512:(tci + 1) * 512], in_=ps[:])
                if n_cores > 1:
                    cc_in = dram.tile([96, TOK], bf16, name="cc_in")
                    cc_out = nc.dram_tensor("cc_out", [96, TOK], bf16, addr_space="Shared")
                    bc_dram = cc_out[R:R + 32, :]
                    nc.sync.dma_start(out=cc_in[:], in_=xdbl_part[:])
                    nc.gpsimd.collective_compute(
                        "AllReduce", ALU.add, replica_groups=[list(range(n_cores))],
                        ins=[cc_in[:].opt()], outs=[cc_out[:].opt()])
                    nc.sync.dma_start(out=dtl_bf[:], in_=cc_out[0:R, :])
                else:
                    bc_dram_t = dram.tile([32, TOK], bf16, name="bc_dram_local")
                    bc_dram = bc_dram_t[:]
                    nc.gpsimd.tensor_copy(out=dtl_bf[:], in_=xdbl_part[0:R, :])
                    nc.sync.dma_start(out=bc_dram, in_=xdbl_part[R:R + 32, :])

                # ========== P4b: in_proj z-half (overlaps the AllReduce) ==========
                for fq in range(NDT, 4):
                    in_proj_fq(fq)

        # ========== P7: dt_proj + softplus; u ==========
        for d in range(NDT):
            for tci in range(4):
                ps = psum.tile([128, 512], f32, tag="mm")
                nc.tensor.matmul(ps[:], dtpw[:, d, :], dtl_bf[:, tci * 512:(tci + 1) * 512],
                                 start=True, stop=True)
                spt = small.tile([128, 512], f32, tag="spt", bufs=2)
                nc.scalar.activation(out=spt[:], in_=ps[:],
                                     func=AF.Exp, bias=dtb[:, d:d + 1])
                nc.scalar.activation(out=dtT[:, d, tci * 512:(tci + 1) * 512], in_=spt[:],
                                     func=AF.Ln, bias=oneb[:])
        for d in range(NDT):
            for tci in range(4):
                nc.gpsimd.tensor_mul(out=uT[:, d, tci * 512:(tci + 1) * 512],
                                     in0=dtT[:, d, tci * 512:(tci + 1) * 512],
                                     in1=xT[:, d, tci * 512:(tci + 1) * 512])

        # ========== P8: pipelined scan units ==========
        # unit = (b, g, d); per unit: dA (scalar), dBx (DVE), scans (Pool),
        # then one C-mult (DVE) + identity-matmul state reduction (PE->PSUM).
        psum_stack.close()
        bcp = ctx.enter_context(tc.tile_pool(name="bcp", bufs=1))
        slabs = ctx.enter_context(tc.tile_pool(name="slabs", bufs=3))
        psum_y = ctx.enter_context(tc.tile_pool(name="psum_y", bufs=1, space="PSUM"))
        op_psum = ctx.enter_context(tc.tile_pool(name="op_psum", bufs=4, space="PSUM"))
        outp = ctx.enter_context(tc.tile_pool(name="outp", bufs=4))

        units = [(b, g, d) for b in range(B) for g in range(NSG) for d in range(NDT)]
        ucount = len(units)
        bc_tiles = {}      # (b, g) -> (Bbc, Cbc)
        slab_tiles = {}    # unit index -> (dA, dBx)
        py_tiles = {}      # (b, d) -> [psum half0, psum half1]

        def src_bc(rows0, b, s0, s1):
            sl = bc_dram[rows0 + s0:rows0 + s1, b * L:(b + 1) * L]
            return sl.rearrange("(o s) t -> o s t", o=1).to_broadcast((128, s1 - s0, L))

        def emit_front(i):
            b, g, d = units[i]
            if (b, g) not in bc_tiles:
                Bbc = bcp.tile([128, SG, L], bf16, tag="Bbc", name=f"Bbc_{b}{g}")
                Cbc = bcp.tile([128, SG, L], bf16, tag="Cbc", name=f"Cbc_{b}{g}")
                for (s0, s1) in ((0, SG // 2), (SG // 2, SG)):
                    nc.sync.dma_start(out=Bbc[:, s0:s1, :], in_=src_bc(g * SG, b, s0, s1))
                    nc.sync.dma_start(out=Cbc[:, s0:s1, :], in_=src_bc(16 + g * SG, b, s0, s1))
                bc_tiles[(b, g)] = (Bbc, Cbc)
            Bbc, _ = bc_tiles[(b, g)]
            dA = slabs.tile([128, SG, L], bf16, tag="dA", name=f"dA_{i}")
            dBx = slabs.tile([128, SG, L], bf16, tag="dBx", name=f"dBx_{i}")
            for s in range(SG):
                nc.scalar.activation(
                    out=dA[:, s, :], in_=dtT[:, d, b * L:(b + 1) * L],
                    func=AF.Exp, scale=Asb[:, d, g * SG + s:g * SG + s + 1])
                nc.gpsimd.tensor_mul(out=dBx[:, s, :],
                                     in0=uT[:, d, b * L:(b + 1) * L],
                                     in1=Bbc[:, s, :])
            for s in range(SG):
                nc.vector.tensor_tensor_scan(out=dA[:, s, :], data0=dA[:, s, :],
                                             data1=dBx[:, s, :], initial=0.0,
                                             op0=ALU.mult, op1=ALU.add)
            slab_tiles[i] = (dA, dBx)

        def emit_back(i):
            b, g, d = units[i]
            sc, _ = slab_tiles.pop(i)
            _, Cbc = bc_tiles[(b, g)]
            nc.vector.tensor_mul(out=sc[:, 0:3, :], in0=sc[:, 0:3, :], in1=Cbc[:, 0:3, :])
            nc.gpsimd.tensor_mul(out=sc[:, 3:8, :], in0=sc[:, 3:8, :], in1=Cbc[:, 3:8, :])
            if (b, d) not in py_tiles:
                py_tiles[(b, d)] = [
                    psum_y.tile([128, 512], f32, tag=f"py{d}{h}", name=f"py_{b}{d}{h}")
                    for h in range(2)]
            py = py_tiles[(b, d)]
            for s in range(SG):
                for h in range(2):
                    nc.tensor.matmul(py[h][:], ident[:],
                                     sc[:, s, h * 512:(h + 1) * 512],
                                     start=(g == 0 and s == 0), stop=False)
            if g == NSG - 1:
                # fold D*x and close the accumulation
                for h in range(2):
                    nc.tensor.matmul(py[h][:], diagD[:, d, :],
                                     xT[:, d, b * L + h * 512:b * L + (h + 1) * 512],
                                     start=False, stop=(h == 1))
                # yT = psum * silu(z)
                nc.vector.tensor_mul(out=yT[:, d, b * L:b * L + 512],
                                     in0=py[0][:],
                                     in1=szT[:, d, b * L:b * L + 512])
                nc.vector.tensor_mul(out=yT[:, d, b * L + 512:(b + 1) * L],
                                     in0=py[1][:],
                                     in1=szT[:, d, b * L + 512:(b + 1) * L])
                py_tiles.pop((b, d))

        op_queue = []

        def emit_outproj_tile(i):
            for nchunk in range(2):
                ps = op_psum.tile([128, 512], f32, tag="op")
                for d in range(NDT):
                    nc.tensor.matmul(ps[:], yT[:, d, i * 128:(i + 1) * 128],
                                     opw[:, d, nchunk * 512:(nchunk + 1) * 512],
                                     start=(d == 0), stop=(d == NDT - 1))
                ot = outp.tile([128, 512], bf16, tag="ot")
                nc.scalar.copy(out=ot[:], in_=ps[:])
                nc.sync.dma_start(out=outv[i][:, nchunk * 512:(nchunk + 1) * 512], in_=ot[:])

        def emit_outproj(b):
            op_queue.extend(range(b * (NTT // B), (b + 1) * (NTT // B)))

        def drain_outproj(k):
            for _ in range(min(k, len(op_queue))):
                emit_outproj_tile(op_queue.pop(0))

        DEPTH = 1
        for i in range(ucount + DEPTH):
            if i < ucount:
                emit_front(i)
            if i >= DEPTH:
                j = i - DEPTH
                emit_back(j)
                bj, gj, dj = units[j]
                if gj == NSG - 1 and dj == NDT - 1:
                    emit_outproj(bj)
            drain_outproj(2)
        drain_outproj(len(op_queue))


def _conv_diag(w):
    """w: [DLOC, KCONV] -> [NDT*KCONV*128, 128] block-diag lhsT layout."""
    out = np.zeros((NDT, KCONV, 128, 128), np.float32)
    for d in range(NDT):
        for k in range(KCONV):
            out[d, k] = np.diag(w[d * 128:(d + 1) * 128, k])
    return out.reshape(NDT * KCONV * 128, 128)


def prep_core_inputs(inputs, core):
    """Host-side weight prep for one core. inputs: raw np arrays from setup_inputs."""
    import ml_dtypes
    bf = ml_dtypes.bfloat16
    sl = slice(core * DLOC, (core + 1) * DLOC)
    ln_w = np.asarray(inputs["ln_w"], np.float32)
    ln_b = np.asarray(inputs["ln_b"], np.float32)
    ipw = np.asarray(inputs["in_proj_w"], np.float32)
    rows = np.concatenate([ipw[sl], ipw[DI + core * DLOC: DI + (core + 1) * DLOC]])  # x|z
    W_fold = rows * ln_w[None, :]
    in_b = rows @ ln_b
    Dv = np.asarray(inputs["D"], np.float32)[sl]
    diagD = np.zeros((DLOC, 128), np.float32)
    for dtl in range(NDT):
        blk = slice(dtl * 128, (dtl + 1) * 128)
        diagD[blk, :] = np.diag(Dv[blk])
    d = {
        "h": np.ascontiguousarray(np.asarray(inputs["h"], np.float32).reshape(TOK, DM)).astype(bf),
        "res": np.ascontiguousarray(np.asarray(inputs["residual"], np.float32).reshape(TOK, DM)).astype(bf),
        "w_in": np.ascontiguousarray(W_fold.T).astype(bf),
        "in_b": in_b.astype(np.float32),
        "convw": _conv_diag(np.asarray(inputs["conv_w"], np.float32)[sl, 0, :]).astype(bf),
        "convb": np.asarray(inputs["conv_b"], np.float32)[sl].copy(),
        "xp": np.ascontiguousarray(np.asarray(inputs["x_proj_w"], np.float32)[:, sl].T).astype(bf),
        "dtp": np.ascontiguousarray(np.asarray(inputs["dt_proj_w"], np.float32)[sl].T).astype(bf),
        "dtb": np.asarray(inputs["dt_proj_b"], np.float32)[sl].copy(),
        "A": (-np.exp(np.asarray(inputs["A_log"], np.float32)[sl])).astype(np.float32),
        "diagD": diagD.astype(bf),
        "op": np.ascontiguousarray(np.asarray(inputs["out_proj_w"], np.float32)[:, sl].T).astype(bf),
        "ident": np.eye(128, dtype=np.float32).astype(bf),
    }
    return d


# ======================= host-side entry point =======================
_CACHE = {}


def _get_nc():
    if "nc" not in _CACHE:
        nc = bass.Bass("TRN2", target_bir_lowering=False, debug=False,
                       num_devices=NCORES, enable_asserts=False)
        build(nc, n_cores=NCORES)
        _CACHE["nc"] = nc
    return _CACHE["nc"]


def kernel(**inputs):
    """Full unsharded inputs (as in reference.setup_inputs()) ->
    (out, residual) as np.float32 arrays of shape (2, 1024, 1024)."""
    from concourse.bass_utils import run_bass_kernel_spmd
    nc = _get_nc()
    inp = {k: np.asarray(v) for k, v in inputs.items()}
    in_maps = [prep_core_inputs(inp, c) for c in range(NCORES)]
    res = run_bass_kernel_spmd(nc, in_maps, core_ids=list(range(NCORES)))
    out = np.zeros((TOK, DM), np.float32)
    for r in res.results:
        out += np.asarray(r["out_part"], dtype=np.float32)
    out = out.reshape(B, L, DM)
    residual = (inp["h"].astype(np.float32) + inp["residual"].astype(np.float32))
    return out, residual


def _make_sharded_runner(nc, in_maps, device_resident=True):
    """jit once; return (fn, args) for repeated timed execution (8-core shard_map)."""
    import jax
    from jax.sharding import Mesh, PartitionSpec, NamedSharding
    from jax.experimental.shard_map import shard_map
    from concourse.bass2jax import _bass_exec_p, install_neuronx_cc_hook, partition_id_tensor
    install_neuronx_cc_hook()
    n_cores = len(in_maps)
    partition_name = nc.partition_id_tensor.name if nc.partition_id_tensor else None
    in_names, out_names, out_avals, zero_outs = [], [], [], []
    for alloc in nc.m.functions[0].allocations:
        if not isinstance(alloc, mybir.MemoryLocationSet):
            continue
        name = alloc.memorylocations[0].name
        if alloc.kind == "ExternalInput":
            if name != partition_name:
                in_names.append(name)
        elif alloc.kind == "ExternalOutput":
            shape = tuple(alloc.tensor_shape)
            dtype = mybir.dt.np(alloc.dtype)
            out_names.append(name)
            out_avals.append(jax.core.ShapedArray(shape, dtype))
            zero_outs.append(np.zeros(shape, dtype))
    all_in = list(in_names) + list(out_names)
    if partition_name is not None:
        all_in.append(partition_name)

    def _body(*args):
        operands = list(args)
        if partition_name is not None:
            operands.append(partition_id_tensor())
        outs = _bass_exec_p.bind(
            *operands, out_avals=tuple(out_avals), in_names=tuple(all_in),
            out_names=tuple(out_names), lowering_input_output_aliases=(),
            sim_require_finite=True, sim_require_nnan=True, nc=nc)
        return tuple(outs)

    devices = jax.devices()[:n_cores]
    mesh = Mesh(np.asarray(devices), ("core",))
    n_params = len(in_names)
    in_specs = (PartitionSpec("core"),) * (n_params + len(out_names))
    out_specs = (PartitionSpec("core"),) * len(out_names)
    fn = jax.jit(shard_map(_body, mesh=mesh, in_specs=in_specs,
                           out_specs=out_specs, check_rep=False), keep_unused=True)
    per_core = [[np.asarray(m[n]) for n in in_names] for m in in_maps]
    concat_in = [np.concatenate([per_core[c][i] for c in range(n_cores)], axis=0)
                 for i in range(n_params)]
    concat_zeros = [np.zeros((n_cores * z.shape[0], *z.shape[1:]), z.dtype)
                    for z in zero_outs]
    args = concat_in + concat_zeros
    if device_resident:
        sh = NamedSharding(mesh, PartitionSpec("core"))
        args = [jax.device_put(a, sh) for a in args]
        jax.block_until_ready(args)
    return fn, args, out_names, out_avals


def _time_runner(fn, args, reps):
    import jax
    r = fn(*args); jax.block_until_ready(r)
    times = []
    for _ in range(reps):
        t0 = time.perf_counter()
        r = fn(*args)
        jax.block_until_ready(r)
        times.append(time.perf_counter() - t0)
    return min(times)


def _baseline_nc():
    nc = bass.Bass("TRN2", target_bir_lowering=False, debug=False,
                   num_devices=NCORES, enable_asserts=False)
    x = nc.dram_tensor("x", [128, 128], f32, kind="ExternalInput")
    y = nc.dram_tensor("y", [128, 128], f32, kind="ExternalOutput")
    with tile.TileContext(nc) as tc:
        with tc.tile_pool(name="p", bufs=1) as pool:
            t = pool.tile([128, 128], f32)
            nc.sync.dma_start(out=t[:], in_=x[:])
            nc.sync.dma_start(out=y[:], in_=t[:])
    split_multiwaits(nc)
    return nc


def measure_exec_ns(inputs, reps=30):
    """Paired/interleaved timing: alternate kernel and empty dispatches so the
    RPC-floor drift cancels; report median of per-pair differences."""
    import jax
    inp = {k: np.asarray(v) for k, v in inputs.items()}
    in_maps = [prep_core_inputs(inp, c) for c in range(NCORES)]
    fn, args, _, _ = _make_sharded_runner(_get_nc(), in_maps)
    bnc = _baseline_nc()
    bmaps = [{"x": np.zeros((128, 128), np.float32)} for _ in range(NCORES)]
    bfn, bargs, _, _ = _make_sharded_runner(bnc, bmaps)
    # warmup both
    jax.block_until_ready(fn(*args))
    jax.block_until_ready(bfn(*bargs))
    diffs = []
    tks, tbs = [], []
    for _ in range(reps):
        t0 = time.perf_counter()
        jax.block_until_ready(bfn(*bargs))
        t1 = time.perf_counter()
        jax.block_until_ready(fn(*args))
        t2 = time.perf_counter()
        tbs.append(t1 - t0)
        tks.append(t2 - t1)
        diffs.append((t2 - t1) - (t1 - t0))
    diffs.sort()
    med = diffs[len(diffs) // 2]
    print(f"  [wall min: kernel {min(tks)*1e3:.2f} ms, empty {min(tbs)*1e3:.2f} ms, "
          f"median paired diff {med*1e3:.3f} ms]")
    return max(med, 0.0) * 1e9


# revision 55
# speedup vs baseline: 1.4737x; 1.4737x over previous
"""Trainium2 Bass kernel for the Mamba-1 block (nn_Block_9122510537354).

Self-contained: hardcodes shapes/sharding. d_inner is sharded over 8 cores;
each core computes a partial out_proj contribution which the host sums.
The kernel returns (out, residual) matching reference.reference().

Design notes (engine assignment chosen against the CoreSim timing model,
within the real TRN2 backend constraints: GPSIMD has no PSUM access and no
tensor_tensor_scan/scalar_tensor_tensor; DMA cannot read PSUM):
- scan section: dA exps on Scalar, dBx mults on GpSimd, the 64 sequential
  scans on DVE (the only engine that can scan), C-mult split DVE/GpSimd,
  and the state-reduction as identity-matmuls accumulated in PSUM on PE
  (replaces the old DVE add tree). D*x is folded in via a diag(D) matmul;
  silu(z) gating is one DVE mult straight out of PSUM.
- depthwise conv runs on PE as 4 shifted diag(w_k) matmuls into PSUM.
- hn transposed to feature-major via SBUF->SBUF DMA transposes on the
  SP/Activation queues (frees PE and removes PSUM copy traffic).
- B/C rows broadcast to 128 partitions with batched DMAs per (b, g)
  reading the AllReduce output in DRAM directly.
- ONE bf16 AllReduce for all 96 x_proj rows (collective count matters far
  more than payload on HW: each extra collective ~450us); the in_proj
  z-half is emitted after the collective launch so PE overlaps it.
- h/res stream in as bf16 (halves input DMA), out partials stored bf16
  and summed in f32 on the host.
- LN: wave-scheduled per 4-tile group into one big h/res buffer with
  in-place slot reuse, fused (rn-mu)*rstd on GpSimd, transposes deferred
  one group to avoid cross-engine dependency ladders on in-order queues.
- software pipeline: unit i+1's dA/dBx/scans are emitted before unit i's
  C-mult; triple-buffered slabs; out_proj drained 2 tiles per unit.
"""
import contextlib
import time
import numpy as np

import concourse.tile as tile
import concourse.mybir as mybir
from concourse.vector_clock import ScopedClock


def _patched_drain_and_barrier(self, tick_clock, wait_clock):
    nc = self.nc
    probe = nc.sync.nop(nofuse=True, hint="drain_waits")
    wait_clock.add_sem_waits(probe.ins, ScopedClock({None: tick_clock.global_clock}))
    waits = list(probe.ins.sync_info.on_wait)
    if len(waits) > 1:
        probe.ins.sync_info.on_wait[:] = waits[:1]
        for w in waits[1:]:
            extra = nc.sync.nop(nofuse=True, hint="drain_waits")
            extra.ins.sync_info = mybir.SyncInfo(on_wait=[w], on_update=[])
    nc.sync.drain()
    nc.all_engine_barrier()
    assert self.sems is not None
    popped = nc._tile_sem_poison_stack.pop()
    assert popped is self._sem_poison
    nc.clear_and_free_semaphores(list(self.sems.allocated().values()))
    nc.all_engine_barrier()


tile.TileContext._drain_and_barrier = _patched_drain_and_barrier

_split_ctr = [0]


def split_multiwaits(nc):
    """Mutate nc.m so no instruction carries >1 sync wait."""
    n_split = 0
    for fn in nc.m.functions:
        for blk in fn.blocks:
            insts = blk.instructions
            i = 0
            while i < len(insts):
                inst = insts[i]
                si = getattr(inst, "sync_info", None)
                if si is not None and si.on_wait and len(si.on_wait) > 1:
                    waits = list(si.on_wait)
                    si.on_wait[:] = waits[:1]
                    new_nops = []
                    for w in waits[1:]:
                        _split_ctr[0] += 1
                        new_nops.append(
                            mybir.InstNoOp(
                                name=f"I-mwsplit-{_split_ctr[0]}",
                                engine=inst.engine,
                                bass_nofuse=True,
                                sync_info=mybir.SyncInfo(on_wait=[w], on_update=[]),
                            )
                        )
                    insts[i:i] = new_nops
                    i += len(new_nops)
                    n_split += 1
                i += 1
    return n_split


import concourse.bass as bass
import concourse.tile as tile
import concourse.mybir as mybir

dt = mybir.dt
AF = mybir.ActivationFunctionType
ALU = mybir.AluOpType

B, L, DM = 2, 1024, 1024
DI, S, R, KCONV = 2048, 16, 64, 4
NCORES = 8
DLOC = DI // NCORES          # 256
TOK = B * L                  # 2048
NTT = TOK // 128             # 16 token tiles
NDT = DLOC // 128            # 2 channel tiles
SG = 8                       # states per group
NSG = S // SG                # 2 groups
EPS = 1e-5
LPAD = L + 4                 # per-b padded conv row

f32, bf16 = dt.float32, dt.bfloat16


def build(nc, n_cores=NCORES):
    # ---------------- DRAM I/O ----------------
    h_d = nc.dram_tensor("h", [TOK, DM], bf16, kind="ExternalInput")
    res_d = nc.dram_tensor("res", [TOK, DM], bf16, kind="ExternalInput")
    W_in_d = nc.dram_tensor("w_in", [DM, 2 * DLOC], bf16, kind="ExternalInput")
    in_b_d = nc.dram_tensor("in_b", [2 * DLOC], f32, kind="ExternalInput")
    convw_d = nc.dram_tensor("convw", [NDT * KCONV * 128, 128], bf16, kind="ExternalInput")
    convb_d = nc.dram_tensor("convb", [DLOC], f32, kind="ExternalInput")
    xp_d = nc.dram_tensor("xp", [DLOC, 96], bf16, kind="ExternalInput")
    dtp_d = nc.dram_tensor("dtp", [R, DLOC], bf16, kind="ExternalInput")
    dtb_d = nc.dram_tensor("dtb", [DLOC], f32, kind="ExternalInput")
    A_d = nc.dram_tensor("A", [DLOC, S], f32, kind="ExternalInput")
    diagD_d = nc.dram_tensor("diagD", [DLOC, 128], bf16, kind="ExternalInput")
    op_d = nc.dram_tensor("op", [DLOC, DM], bf16, kind="ExternalInput")
    ident_d = nc.dram_tensor("ident", [128, 128], bf16, kind="ExternalInput")

    out_d = nc.dram_tensor("out_part", [TOK, DM], bf16, kind="ExternalOutput")

    hv = h_d[:].rearrange("(n p) d -> n p d", p=128)
    rv = res_d[:].rearrange("(n p) d -> n p d", p=128)
    outv = out_d[:].rearrange("(n p) d -> n p d", p=128)

    _res = _build_body(nc, locals())
    split_multiwaits(nc)
    return _res


def _build_body(nc, T):
    h_d, res_d, W_in_d, in_b_d = T["h_d"], T["res_d"], T["W_in_d"], T["in_b_d"]
    convw_d, convb_d, xp_d, dtp_d = T["convw_d"], T["convb_d"], T["xp_d"], T["dtp_d"]
    dtb_d, A_d, diagD_d, op_d, ident_d = T["dtb_d"], T["A_d"], T["diagD_d"], T["op_d"], T["ident_d"]
    out_d, hv, rv, outv = T["out_d"], T["hv"], T["rv"], T["outv"]
    n_cores = T["n_cores"]
    with tile.TileContext(nc) as tc, contextlib.ExitStack() as ctx:
        const = ctx.enter_context(tc.tile_pool(name="const", bufs=1))
        live = ctx.enter_context(tc.tile_pool(name="live", bufs=1))
        small = ctx.enter_context(tc.tile_pool(name="small", bufs=6))
        dram = ctx.enter_context(tc.tile_pool(name="dram", bufs=1, space="DRAM"))

        # ---------------- constants ----------------
        in_bias = const.tile([128, (2 * DLOC) // 128], f32)
        nc.sync.dma_start(out=in_bias[:], in_=in_b_d[:].rearrange("(ft p) -> p ft", p=128))
        convdiag = const.tile([128, NDT, KCONV, 128], bf16)
        nc.scalar.dma_start(out=convdiag[:],
                            in_=convw_d[:].rearrange("(dtl k p) f -> p dtl k f", p=128, k=KCONV))
        convb = const.tile([128, NDT], f32)
        nc.sync.dma_start(out=convb[:], in_=convb_d[:].rearrange("(dtl p) -> p dtl", p=128))
        xpw = const.tile([128, NDT, 96], bf16)
        nc.scalar.dma_start(out=xpw[:], in_=xp_d[:].rearrange("(dtl p) f -> p dtl f", p=128))
        dtpw = const.tile([R, NDT, 128], bf16)
        nc.scalar.dma_start(out=dtpw[:], in_=dtp_d[:].rearrange("r (dtl p) -> r dtl p", p=128))
        dtb = const.tile([128, NDT], f32)
        nc.sync.dma_start(out=dtb[:], in_=dtb_d[:].rearrange("(dtl p) -> p dtl", p=128))
        Asb = const.tile([128, NDT, S], f32)
        nc.sync.dma_start(out=Asb[:], in_=A_d[:].rearrange("(dtl p) s -> p dtl s", p=128))
        diagD = const.tile([128, NDT, 128], bf16)
        nc.scalar.dma_start(out=diagD[:], in_=diagD_d[:].rearrange("(dtl p) f -> p dtl f", p=128))
        opw = const.tile([128, NDT, DM], bf16)
        nc.scalar.dma_start(out=opw[:], in_=op_d[:].rearrange("(dtl p) f -> p dtl f", p=128))
        ident = const.tile([128, 128], bf16)
        nc.scalar.dma_start(out=ident[:], in_=ident_d[:])
        epsb = const.tile([128, 1], f32)
        nc.vector.memset(epsb[:], EPS)
        oneb = const.tile([128, 1], f32)
        nc.vector.memset(oneb[:], 1.0)

        # persistent feature-major buffers
        szT = live.tile([128, NDT, TOK], bf16, tag="szT")
        xT = live.tile([128, NDT, TOK], bf16, tag="xT")
        dtT = live.tile([128, NDT, TOK], bf16, tag="dtT")
        uT = live.tile([128, NDT, TOK], bf16, tag="uT")
        yT = live.tile([128, NDT, TOK], bf16, tag="yT")
        dtl_bf = live.tile([R, TOK], bf16, tag="dtl_bf")

        # ========== P1-P3: add + LN + normalize + transpose ==========
        with tc.tile_pool(name="scope13", bufs=1) as scope13:
            hnT = scope13.tile([128, DM // 128, TOK], bf16, tag="hnT")
            W_in = scope13.tile([128, DM // 128, 2 * DLOC], bf16, tag="W_in")
            tq = [nc.sync, nc.scalar]
            hres = scope13.tile([128, NTT, 2, DM], bf16, tag="hres")
            for i in range(4):
                nc.sync.dma_start(out=hres[:, i, 0, :], in_=hv[i])
                nc.sync.dma_start(out=hres[:, i, 1, :], in_=rv[i])
            nc.sync.dma_start(out=W_in[:], in_=W_in_d[:].rearrange("(kt p) f -> p kt f", p=128))

            def load_chunk(c):
                nc.sync.dma_start(
                    out=hres[:, 4 * c:4 * (c + 1), 0, :],
                    in_=h_d[:].rearrange("(n p) d -> n p d", p=128)[4 * c:4 * (c + 1)]
                        .rearrange("n p d -> p n d"))
                nc.scalar.dma_start(
                    out=hres[:, 4 * c:4 * (c + 1), 1, :],
                    in_=res_d[:].rearrange("(n p) d -> n p d", p=128)[4 * c:4 * (c + 1)]
                        .rearrange("n p d -> p n d"))

            load_chunk(1)
            pending_tdma = []

            def flush_tdma():
                for i in pending_tdma:
                    for j in range(8):
                        eng = nc.scalar if i < 4 else tq[(i * 8 + j) % 2]
                        eng.dma_start(
                            out=hnT[:, j, i * 128:(i + 1) * 128],
                            in_=hres[:, i, 0, j * 128:(j + 1) * 128], transpose=True)
                pending_tdma.clear()

            for grp in range(NTT // 4):
                gtiles = list(range(grp * 4, (grp + 1) * 4))
                rstds = {}
                for i in gtiles:
                    nc.gpsimd.tensor_add(out=hres[:, i, 1, :], in0=hres[:, i, 0, :],
                                         in1=hres[:, i, 1, :])
                    st = small.tile([128, 2, 6], f32, tag=f"st{i % 8}")
                    nc.vector.bn_stats(out=st[:, 0, :], in_=hres[:, i, 1, 0:512])
                    nc.vector.bn_stats(out=st[:, 1, :], in_=hres[:, i, 1, 512:1024])
                    mv = small.tile([128, 2], f32, tag=f"mv{i % 8}")
                    nc.vector.bn_aggr(out=mv[:], in_=st[:])
                    rstds[i] = mv
                for i in gtiles:
                    mv = rstds[i]
                    rstd = small.tile([128, 1], f32, tag=f"rstd{i % 8}")
                    nc.scalar.activation(out=rstd[:], in_=mv[:, 1:2], func=AF.Sqrt, bias=epsb[:])
                    rstds[i] = (mv, rstd)
                for i in gtiles:
                    _, rstd = rstds[i]
                    nc.vector.reciprocal(out=rstd[:], in_=rstd[:])
                for i in gtiles:
                    mv, rstd = rstds[i]
                    nc.gpsimd.tensor_scalar(out=hres[:, i, 0, :], in0=hres[:, i, 1, :],
                                            scalar1=mv[:, 0:1], scalar2=rstd[:],
                                            op0=ALU.subtract, op1=ALU.mult)
                prev = pending_tdma[:]
                pending_tdma.clear()
                pending_tdma.extend(gtiles)
                for i in prev:
                    for j in range(8):
                        eng = nc.scalar if i < 4 else tq[(i * 8 + j) % 2]
                        eng.dma_start(
                            out=hnT[:, j, i * 128:(i + 1) * 128],
                            in_=hres[:, i, 0, j * 128:(j + 1) * 128], transpose=True)
                if grp + 2 < 4:
                    load_chunk(grp + 2)
            flush_tdma()

            psum_stack = contextlib.ExitStack()
            psum = psum_stack.enter_context(tc.tile_pool(name="psum", bufs=2, space="PSUM"))
            # ========== P4a: in_proj x-half ==========
            with tc.tile_pool(name="scope45", bufs=1) as scope45:
                xpreT = scope45.tile([128, NDT, B, LPAD], bf16, tag="xpreT")
                for d in range(NDT):
                    for b in range(B):
                        nc.vector.memset(xpreT[:, d, b, 0:4], 0.0)

                def in_proj_fq(fq):
                    for tci in range(4):   # token chunks of 512; 0,1 -> b0; 2,3 -> b1
                        b, off = divmod(tci, 2)
                        ps = psum.tile([128, 512], f32, tag="mm")
                        for k in range(8):
                            nc.tensor.matmul(ps[:], W_in[:, k, fq * 128:(fq + 1) * 128],
                                             hnT[:, k, tci * 512:(tci + 1) * 512],
                                             start=(k == 0), stop=(k == 7))
                        if fq < NDT:
                            nc.scalar.activation(
                                out=xpreT[:, fq, b, 4 + off * 512:4 + (off + 1) * 512],
                                in_=ps[:], func=AF.Identity, bias=in_bias[:, fq:fq + 1])
                        else:
                            nc.scalar.activation(
                                out=szT[:, fq - NDT, tci * 512:(tci + 1) * 512],
                                in_=ps[:], func=AF.Silu, bias=in_bias[:, fq:fq + 1])

                for fq in range(NDT):
                    in_proj_fq(fq)
                # ========== P5: conv (PE diag matmuls) + silu ==========
                for d in range(NDT):
                    for b in range(B):
                        for h in range(2):
                            cps = psum.tile([128, 512], f32, tag="convp")
                            for k in range(KCONV):
                                nc.tensor.matmul(
                                    cps[:], convdiag[:, d, k, :],
                                    xpreT[:, d, b, 1 + k + h * 512:1 + k + h * 512 + 512],
                                    start=(k == 0), stop=(k == KCONV - 1))
                            nc.scalar.activation(
                                out=xT[:, d, b * L + h * 512:b * L + (h + 1) * 512],
                                in_=cps[:], func=AF.Silu, bias=convb[:, d:d + 1])

                # ========== P6: x_proj partial + AllReduce (launch) ==========
                xdbl_part = scope45.tile([96, TOK], bf16, tag="xdblp")
                for tci in range(4):
                    ps = psum.tile([96, 512], f32, tag="xdbl")
                    for d in range(NDT):
                        nc.tensor.matmul(ps[:], xpw[:, d, :], xT[:, d, tci * 512:(tci + 1) * 512],
                                         start=(d == 0), stop=(d == NDT - 1))
                    nc.vector.tensor_copy(out=xdbl_part[:, tci * 512:(tci + 1) * 512], in_=ps[:])
                if n_cores > 1:
                    cc_in = dram.tile([96, TOK], bf16, name="cc_in")
                    cc_out = nc.dram_tensor("cc_out", [96, TOK], bf16, addr_space="Shared")
                    bc_dram = cc_out[R:R + 32, :]
                    nc.sync.dma_start(out=cc_in[:], in_=xdbl_part[:])
                    nc.gpsimd.collective_compute(
                        "AllReduce", ALU.add, replica_groups=[list(range(n_cores))],
                        ins=[cc_in[:].opt()], outs=[cc_out[:].opt()])
                    nc.sync.dma_start(out=dtl_bf[:], in_=cc_out[0:R, :])
                else:
                    bc_dram_t = dram.tile([32, TOK], bf16, name="bc_dram_local")
                    bc_dram = bc_dram_t[:]
                    nc.gpsimd.tensor_copy(out=dtl_bf[:], in_=xdbl_part[0:R, :])
                    nc.sync.dma_start(out=bc_dram, in_=xdbl_part[R:R + 32, :])

                # ========== P4b: in_proj z-half (overlaps the AllReduce) ==========
                for fq in range(NDT, 4):
                    in_proj_fq(fq)

        # ========== P7: dt_proj + softplus; u ==========
        for d in range(NDT):
            for tci in range(4):
                ps = psum.tile([128, 512], f32, tag="mm")
                nc.tensor.matmul(ps[:], dtpw[:, d, :], dtl_bf[:, tci * 512:(tci + 1) * 512],
                                 start=True, stop=True)
                spt = small.tile([128, 512], f32, tag="spt", bufs=2)
                nc.scalar.activation(out=spt[:], in_=ps[:],
                                     func=AF.Exp, bias=dtb[:, d:d + 1])
                nc.scalar.activation(out=dtT[:, d, tci * 512:(tci + 1) * 512], in_=spt[:],
                                     func=AF.Ln, bias=oneb[:])
        for d in range(NDT):
            for tci in range(4):
                nc.gpsimd.tensor_mul(out=uT[:, d, tci * 512:(tci + 1) * 512],
                                     in0=dtT[:, d, tci * 512:(tci + 1) * 512],
                                     in1=xT[:, d, tci * 512:(tci + 1) * 512])

        # ========== P8: pipelined scan units ==========
        # unit = (b, g, d); per unit: dA (scalar), dBx (DVE), scans (Pool),
        # then one C-mult (DVE) + identity-matmul state reduction (PE->PSUM).
        psum_stack.close()
        bcp = ctx.enter_context(tc.tile_pool(name="bcp", bufs=1))
        slabs = ctx.enter_context(tc.tile_pool(name="slabs", bufs=3))
        psum_y = ctx.enter_context(tc.tile_pool(name="psum_y", bufs=1, space="PSUM"))
        op_psum = ctx.enter_context(tc.tile_pool(name="op_psum", bufs=4, space="PSUM"))
        outp = ctx.enter_context(tc.tile_pool(name="outp", bufs=4))

        units = [(b, g, d) for b in range(B) for g in range(NSG) for d in range(NDT)]
        ucount = len(units)
        bc_tiles = {}      # (b, g) -> (Bbc, Cbc)
        slab_tiles = {}    # unit index -> (dA, dBx)
        py_tiles = {}      # (b, d) -> [psum half0, psum half1]

        def src_bc(rows0, b, s0, s1):
            sl = bc_dram[rows0 + s0:rows0 + s1, b * L:(b + 1) * L]
            return sl.rearrange("(o s) t -> o s t", o=1).to_broadcast((128, s1 - s0, L))

        def emit_front(i):
            b, g, d = units[i]
            if (b, g) not in bc_tiles:
                Bbc = bcp.tile([128, SG, L], bf16, tag="Bbc", name=f"Bbc_{b}{g}")
                Cbc = bcp.tile([128, SG, L], bf16, tag="Cbc", name=f"Cbc_{b}{g}")
                for (s0, s1) in ((0, SG // 2), (SG // 2, SG)):
                    nc.sync.dma_start(out=Bbc[:, s0:s1, :], in_=src_bc(g * SG, b, s0, s1))
                    nc.sync.dma_start(out=Cbc[:, s0:s1, :], in_=src_bc(16 + g * SG, b, s0, s1))
                bc_tiles[(b, g)] = (Bbc, Cbc)
            Bbc, _ = bc_tiles[(b, g)]
            dA = slabs.tile([128, SG, L], bf16, tag="dA", name=f"dA_{i}")
            dBx = slabs.tile([128, SG, L], bf16, tag="dBx", name=f"dBx_{i}")
            for s in range(SG):
                nc.scalar.activation(
                    out=dA[:, s, :], in_=dtT[:, d, b * L:(b + 1) * L],
                    func=AF.Exp, scale=Asb[:, d, g * SG + s:g * SG + s + 1])
                nc.gpsimd.tensor_mul(out=dBx[:, s, :],
                                     in0=uT[:, d, b * L:(b + 1) * L],
                                     in1=Bbc[:, s, :])
            for s in range(SG):
                nc.vector.tensor_tensor_scan(out=dA[:, s, :], data0=dA[:, s, :],
                                             data1=dBx[:, s, :], initial=0.0,
                                             op0=ALU.mult, op1=ALU.add)
            slab_tiles[i] = (dA, dBx)

        def emit_back(i):
            b, g, d = units[i]
            sc, _ = slab_tiles.pop(i)
            _, Cbc = bc_tiles[(b, g)]
            nc.vector.tensor_mul(out=sc[:, 0:3, :], in0=sc[:, 0:3, :], in1=Cbc[:, 0:3, :])
            nc.gpsimd.tensor_mul(out=sc[:, 3:8, :], in0=sc[:, 3:8, :], in1=Cbc[:, 3:8, :])
            if (b, d) not in py_tiles:
                py_tiles[(b, d)] = [
                    psum_y.tile([128, 512], f32, tag=f"py{d}{h}", name=f"py_{b}{d}{h}")
                    for h in range(2)]
            py = py_tiles[(b, d)]
            for s in range(SG):
                for h in range(2):
                    nc.tensor.matmul(py[h][:], ident[:],
                                     sc[:, s, h * 512:(h + 1) * 512],
                                     start=(g == 0 and s == 0), stop=False)
            if g == NSG - 1:
                # fold D*x and close the accumulation
                for h in range(2):
                    nc.tensor.matmul(py[h][:], diagD[:, d, :],
                                     xT[:, d, b * L + h * 512:b * L + (h + 1) * 512],
                                     start=False, stop=(h == 1))
                # yT = psum * silu(z)
                nc.vector.tensor_mul(out=yT[:, d, b * L:b * L + 512],
                                     in0=py[0][:],
                                     in1=szT[:, d, b * L:b * L + 512])
                nc.vector.tensor_mul(out=yT[:, d, b * L + 512:(b + 1) * L],
                                     in0=py[1][:],
                                     in1=szT[:, d, b * L + 512:(b + 1) * L])
                py_tiles.pop((b, d))

        op_queue = []

        def emit_outproj_tile(i):
            for nchunk in range(2):
                ps = op_psum.tile([128, 512], f32, tag="op")
                for d in range(NDT):
                    nc.tensor.matmul(ps[:], yT[:, d, i * 128:(i + 1) * 128],
                                     opw[:, d, nchunk * 512:(nchunk + 1) * 512],
                                     start=(d == 0), stop=(d == NDT - 1))
                ot = outp.tile([128, 512], bf16, tag="ot")
                nc.scalar.copy(out=ot[:], in_=ps[:])
                nc.sync.dma_start(out=outv[i][:, nchunk * 512:(nchunk + 1) * 512], in_=ot[:])

        def emit_outproj(b):
            op_queue.extend(range(b * (NTT // B), (b + 1) * (NTT // B)))

        def drain_outproj(k):
            for _ in range(min(k, len(op_queue))):
                emit_outproj_tile(op_queue.pop(0))

        DEPTH = 1
        for i in range(ucount + DEPTH):
            if i < ucount:
                emit_front(i)
            if i >= DEPTH:
                j = i - DEPTH
                emit_back(j)
                bj, gj, dj = units[j]
                if gj == NSG - 1 and dj == NDT - 1:
                    emit_outproj(bj)
            drain_outproj(2)
        drain_outproj(len(op_queue))


def _conv_diag(w):
    """w: [DLOC, KCONV] -> [NDT*KCONV*128, 128] block-diag lhsT layout."""
    out = np.zeros((NDT, KCONV, 128, 128), np.float32)
    for d in range(NDT):
        for k in range(KCONV):
            out[d, k] = np.diag(w[d * 128:(d + 1) * 128, k])
    return out.reshape(NDT * KCONV * 128, 128)


def prep_core_inputs(inputs, core):
    """Host-side weight prep for one core. inputs: raw np arrays from setup_inputs."""
    import ml_dtypes
    bf = ml_dtypes.bfloat16
    sl = slice(core * DLOC, (core + 1) * DLOC)
    ln_w = np.asarray(inputs["ln_w"], np.float32)
    ln_b = np.asarray(inputs["ln_b"], np.float32)
    ipw = np.asarray(inputs["in_proj_w"], np.float32)
    rows = np.concatenate([ipw[sl], ipw[DI + core * DLOC: DI + (core + 1) * DLOC]])  # x|z
    W_fold = rows * ln_w[None, :]
    in_b = rows @ ln_b
    Dv = np.asarray(inputs["D"], np.float32)[sl]
    diagD = np.zeros((DLOC, 128), np.float32)
    for dtl in range(NDT):
        blk = slice(dtl * 128, (dtl + 1) * 128)
        diagD[blk, :] = np.diag(Dv[blk])
    d = {
        "h": np.ascontiguousarray(np.asarray(inputs["h"], np.float32).reshape(TOK, DM)).astype(bf),
        "res": np.ascontiguousarray(np.asarray(inputs["residual"], np.float32).reshape(TOK, DM)).astype(bf),
        "w_in": np.ascontiguousarray(W_fold.T).astype(bf),
        "in_b": in_b.astype(np.float32),
        "convw": _conv_diag(np.asarray(inputs["conv_w"], np.float32)[sl, 0, :]).astype(bf),
        "convb": np.asarray(inputs["conv_b"], np.float32)[sl].copy(),
        "xp": np.ascontiguousarray(np.asarray(inputs["x_proj_w"], np.float32)[:, sl].T).astype(bf),
        "dtp": np.ascontiguousarray(np.asarray(inputs["dt_proj_w"], np.float32)[sl].T).astype(bf),
        "dtb": np.asarray(inputs["dt_proj_b"], np.float32)[sl].copy(),
        "A": (-np.exp(np.asarray(inputs["A_log"], np.float32)[sl])).astype(np.float32),
        "diagD": diagD.astype(bf),
        "op": np.ascontiguousarray(np.asarray(inputs["out_proj_w"], np.float32)[:, sl].T).astype(bf),
        "ident": np.eye(128, dtype=np.float32).astype(bf),
    }
    return d


# ======================= host-side entry point =======================
_CACHE = {}


def _get_nc():
    if "nc" not in _CACHE:
        nc = bass.Bass("TRN2", target_bir_lowering=False, debug=False,
                       num_devices=NCORES, enable_asserts=False)
        build(nc, n_cores=NCORES)
        _CACHE["nc"] = nc
    return _CACHE["nc"]


def kernel(**inputs):
    """Full unsharded inputs (as in reference.setup_inputs()) ->
    (out, residual) as np.float32 arrays of shape (2, 1024, 1024)."""
    from concourse.bass_utils import run_bass_kernel_spmd
    nc = _get_nc()
    inp = {k: np.asarray(v) for k, v in inputs.items()}
    in_maps = [prep_core_inputs(inp, c) for c in range(NCORES)]
    res = run_bass_kernel_spmd(nc, in_maps, core_ids=list(range(NCORES)))
    out = np.zeros((TOK, DM), np.float32)
    for r in res.results:
        out += np.asarray(r["out_part"], dtype=np.float32)
    out = out.reshape(B, L, DM)
    residual = (inp["h"].astype(np.float32) + inp["residual"].astype(np.float32))
    return out, residual


def _make_sharded_runner(nc, in_maps, device_resident=True):
    """jit once; return (fn, args) for repeated timed execution (8-core shard_map)."""
    import jax
    from jax.sharding import Mesh, PartitionSpec, NamedSharding
    from jax.experimental.shard_map import shard_map
    from concourse.bass2jax import _bass_exec_p, install_neuronx_cc_hook, partition_id_tensor
    install_neuronx_cc_hook()
    n_cores = len(in_maps)
    partition_name = nc.partition_id_tensor.name if nc.partition_id_tensor else None
    in_names, out_names, out_avals, zero_outs = [], [], [], []
    for alloc in nc.m.functions[0].allocations:
        if not isinstance(alloc, mybir.MemoryLocationSet):
            continue
        name = alloc.memorylocations[0].name
        if alloc.kind == "ExternalInput":
            if name != partition_name:
                in_names.append(name)
        elif alloc.kind == "ExternalOutput":
            shape = tuple(alloc.tensor_shape)
            dtype = mybir.dt.np(alloc.dtype)
            out_names.append(name)
            out_avals.append(jax.core.ShapedArray(shape, dtype))
            zero_outs.append(np.zeros(shape, dtype))
    all_in = list(in_names) + list(out_names)
    if partition_name is not None:
        all_in.append(partition_name)

    def _body(*args):
        operands = list(args)
        if partition_name is not None:
            operands.append(partition_id_tensor())
        outs = _bass_exec_p.bind(
            *operands, out_avals=tuple(out_avals), in_names=tuple(all_in),
            out_names=tuple(out_names), lowering_input_output_aliases=(),
            sim_require_finite=True, sim_require_nnan=True, nc=nc)
        return tuple(outs)

    devices = jax.devices()[:n_cores]
    mesh = Mesh(np.asarray(devices), ("core",))
    n_params = len(in_names)
    in_specs = (PartitionSpec("core"),) * (n_params + len(out_names))
    out_specs = (PartitionSpec("core"),) * len(out_names)
    fn = jax.jit(shard_map(_body, mesh=mesh, in_specs=in_specs,
                           out_specs=out_specs, check_rep=False), keep_unused=True)
    per_core = [[np.asarray(m[n]) for n in in_names] for m in in_maps]
    concat_in = [np.concatenate([per_core[c][i] for c in range(n_cores)], axis=0)
                 for i in range(n_params)]
    concat_zeros = [np.zeros((n_cores * z.shape[0], *z.shape[1:]), z.dtype)
                    for z in zero_outs]
    args = concat_in + concat_zeros
    if device_resident:
        sh = NamedSharding(mesh, PartitionSpec("core"))
        args = [jax.device_put(a, sh) for a in args]
        jax.block_until_ready(args)
    return fn, args, out_names, out_avals


def _time_runner(fn, args, reps):
    import jax
    r = fn(*args); jax.block_until_ready(r)
    times = []
    for _ in range(reps):
        t0 = time.perf_counter()
        r = fn(*args)
        jax.block_until_ready(r)
        times.append(time.perf_counter() - t0)
    return min(times)


def _baseline_nc():
    nc = bass.Bass("TRN2", target_bir_lowering=False, debug=False,
                   num_devices=NCORES, enable_asserts=False)
    x = nc.dram_tensor("x", [128, 128], f32, kind="ExternalInput")
    y = nc.dram_tensor("y", [128, 128], f32, kind="ExternalOutput")
    with tile.TileContext(nc) as tc:
        with tc.tile_pool(name="p", bufs=1) as pool:
            t = pool.tile([128, 128], f32)
            nc.sync.dma_start(out=t[:], in_=x[:])
            nc.sync.dma_start(out=y[:], in_=t[:])
    split_multiwaits(nc)
    return nc


def measure_exec_ns(inputs, reps=30):
    """Paired/interleaved timing: alternate kernel and empty dispatches so the
    RPC-floor drift cancels; report median of per-pair differences."""
    import jax
    inp = {k: np.asarray(v) for k, v in inputs.items()}
    in_maps = [prep_core_inputs(inp, c) for c in range(NCORES)]
    fn, args, _, _ = _make_sharded_runner(_get_nc(), in_maps)
    bnc = _baseline_nc()
    bmaps = [{"x": np.zeros((128, 128), np.float32)} for _ in range(NCORES)]
    bfn, bargs, _, _ = _make_sharded_runner(bnc, bmaps)
    # warmup both
    jax.block_until_ready(fn(*args))
    jax.block_until_ready(bfn(*bargs))
    diffs = []
    tks, tbs = [], []
    for _ in range(reps):
        t0 = time.perf_counter()
        jax.block_until_ready(bfn(*bargs))
        t1 = time.perf_counter()
        jax.block_until_ready(fn(*args))
        t2 = time.perf_counter()
        tbs.append(t1 - t0)
        tks.append(t2 - t1)
        diffs.append((t2 - t1) - (t1 - t0))
    diffs.sort()
    med = diffs[len(diffs) // 2]
    print(f"  [wall min: kernel {min(tks)*1e3:.2f} ms, empty {min(tbs)*1e3:.2f} ms, "
          f"median paired diff {med*1e3:.3f} ms]")
    return max(med, 0.0) * 1e9
